# revision 12
# baseline (speedup 1.0000x reference)
"""Trainium2 Bass kernel for nn_AugmentPipe — quad-gather version.

v1 bottleneck: 2156 indirect-DMA gathers (1 pixel each) x ~1.1us serial gpsimd
descriptor-gen = 2.4ms. v2: one descriptor serves FOUR consecutive output
pixels along a row: table cell (m, t) holds [3ch x 4 horizontal neighbors
m..m+3] (12 f16 values); a 6-cell contiguous run (72 values) covers the 2x2
stencils of a whole quad. Gathers: 541 instructions instead of 2156.

Feasibility per sample (quad u-span <= 2, v-span <= 4) is guaranteed by a
per-sample host-side D4 normalization: optionally transpose the OUTPUT
traversal (gather the transposed output, un-transpose on host after the
symmetric downsample) and/or transpose the SOURCE image (build the table
from img^T). Both are exact symmetries of the separable pipeline.

Combine: per-pixel 24-slot weight vector (6 cells x 4 majors, 4 nonzeros at
host-computed offsets); DVE broadcast-multiply + tensor_reduce(XY).
"""

import math
import os
import sys

for _p in ("/opt/trn_rl_repo", "/root/.axon_site/_ro/trn_rl_repo"):
    if os.path.isdir(_p) and _p not in sys.path:
        sys.path.append(_p)

import numpy as np

import concourse.bass as bass
import concourse.bacc as bacc
import concourse.mybir as mybir
from concourse.tile import TileContext
from concourse.masks import make_identity

F32 = np.float32
F16 = np.float16

CONV_MODE = "f16"
TBL_MODE = "f16"

N_CORES = 8
TAPS = 12
H = W = 256
HO = WO = 524
HUP = WUP = 1532
QX = WO // 4          # 131 quads per output row
YTM = 4               # full 128-row output tiles
TY = HO - YTM * 128   # 12 tail rows
TGRP = 4              # tail x-groups (32-partition aligned)
TK = 33               # tail quads per group (4*33=132 >= 131)
TP = 128              # tail partitions: p = g*32 + yl, yl < TY used
NCELL = 6             # cells fetched per quad
NMAJ = 4              # majors per cell
NSLOT = NCELL * NMAJ  # 24


# ----------------------------------------------------------------------------
# host-side math
# ----------------------------------------------------------------------------

def _compose_G(u_flip, u_rot90, u_int, z_scale, u_rot1, z_aniso, u_rot2, z_frac):
    n = u_flip.shape[0]
    pi = F32(np.pi)

    def s2(sx, sy):
        m = np.zeros((n, 3, 3), F32)
        m[:, 0, 0] = sx; m[:, 1, 1] = sy; m[:, 2, 2] = 1
        return m

    def t2(tx, ty):
        m = np.zeros((n, 3, 3), F32)
        m[:, 0, 0] = 1; m[:, 1, 1] = 1; m[:, 2, 2] = 1
        m[:, 0, 2] = tx; m[:, 1, 2] = ty
        return m

    def r2(theta):
        c = np.cos(theta, dtype=F32); s = np.sin(theta, dtype=F32)
        m = np.zeros((n, 3, 3), F32)
        m[:, 0, 0] = c; m[:, 0, 1] = -s
        m[:, 1, 0] = s; m[:, 1, 1] = c
        m[:, 2, 2] = 1
        return m

    XINT_MAX, SCALE_STD, ROTATE_MAX, ANISO_STD, XFRAC_STD = 0.125, 0.2, 1.0, 0.2, 0.125
    i_f = np.floor(u_flip * 2).astype(F32)
    G = s2((1.0 / (1.0 - 2.0 * i_f)).astype(F32), np.ones(n, F32))
    i_r = np.floor(u_rot90 * 4).astype(F32)
    G = G @ r2((pi / 2 * i_r).astype(F32))
    t = ((u_int * 2 - 1) * F32(XINT_MAX)).astype(F32)
    G = G @ t2(-np.round(t[:, 0] * W).astype(F32), -np.round(t[:, 1] * H).astype(F32))
    s = np.exp2(z_scale * F32(SCALE_STD)).astype(F32)
    G = G @ s2((1.0 / s).astype(F32), (1.0 / s).astype(F32))
    th1 = ((u_rot1 * 2 - 1) * pi * F32(ROTATE_MAX)).astype(F32)
    G = G @ r2(th1)
    sa = np.exp2(z_aniso * F32(ANISO_STD)).astype(F32)
    G = G @ s2((1.0 / sa).astype(F32), sa)
    th2 = ((u_rot2 * 2 - 1) * pi * F32(ROTATE_MAX)).astype(F32)
    G = G @ r2(th2)
    tf = (z_frac * F32(XFRAC_STD)).astype(F32)
    G = G @ t2((-tf[:, 0] * W).astype(F32), (-tf[:, 1] * H).astype(F32))

    def cs(sx, sy):
        return np.array([[sx, 0, 0], [0, sy, 0], [0, 0, 1]], F32)

    def ct(tx, ty):
        return np.array([[1, 0, tx], [0, 1, ty], [0, 0, 1]], F32)

    G = cs(2, 2)[None] @ G @ cs(0.5, 0.5)[None]
    G = ct(-0.5, -0.5)[None] @ G @ ct(0.5, 0.5)[None]
    G = cs(2.0 / WUP, 2.0 / HUP)[None] @ G @ cs(WO / 2.0, HO / 2.0)[None]
    return G.astype(F32)


def _fields(th):
    """ix, iy sample coords [HO, WO] in upsampled pixels."""
    xs = ((np.arange(WO, dtype=F32) + F32(0.5)) * F32(2.0 / WO) - F32(1.0)).astype(F32)
    ys = ((np.arange(HO, dtype=F32) + F32(0.5)) * F32(2.0 / HO) - F32(1.0)).astype(F32)
    gx = (th[0, 0] * xs[None, :] + th[0, 1] * ys[:, None] + th[0, 2]).astype(F32)
    gy = (th[1, 0] * xs[None, :] + th[1, 1] * ys[:, None] + th[1, 2]).astype(F32)
    ix = ((gx + F32(1.0)) * F32(WUP * 0.5) - F32(0.5)).astype(F32)
    iy = ((gy + F32(1.0)) * F32(HUP * 0.5) - F32(0.5)).astype(F32)
    return ix, iy


def _upsample_band(org, end, size_up, f):
    taps = f.shape[0]
    vr = np.arange(org, end)
    m_lo = np.ceil((vr + 5 - (taps - 1)) / 2).astype(int)
    m_hi = np.floor((vr + 5) / 2).astype(int)
    j_org = max(0, int(m_lo.min()))
    j_end = int(m_hi.max()) + 1
    B = np.zeros((j_end - j_org, end - org), F32)
    valid = (vr >= 0) & (vr < size_up)
    for i in np.nonzero(valid)[0]:
        v = vr[i]
        for m in range(max(m_lo[i], 0), min(m_hi[i], j_end - 1) + 1):
            k = v + 5 - 2 * m
            if 0 <= k < taps:
                B[m - j_org, i] = f[k]
    return B, j_org, j_end


def _downsample_band(n_out, n_in, f):
    D = np.zeros((n_in, n_out), F32)
    for Y in range(n_out):
        for j in range(f.shape[0]):
            y = 2 * Y + 1 + j
            if 0 <= y < n_in:
                D[y, Y] = f[j]
    return D


def _quad_spans(a, valid):
    """per-quad (min, max) over valid pixels of int array a [HO, WO]."""
    aq = a.reshape(HO, QX, 4)
    vq = valid.reshape(HO, QX, 4)
    big = np.int64(1) << 40
    mn = np.where(vq, aq, big).min(axis=2)
    mx = np.where(vq, aq, -big).max(axis=2)
    return mn, mx, vq.any(axis=2)


def _plan_sample(th):
    """Pick D4 transform; return per-sample plan dict."""
    ix, iy = _fields(th)
    best = None
    for otr in (0, 1):
        for st in (0, 1):
            iu, iv = (ix, iy) if st == 0 else (iy, ix)   # st: transpose source
            if otr:                                       # transpose output
                iu = np.ascontiguousarray(iu.T)
                iv = np.ascontiguousarray(iv.T)
            u0f = np.floor(iu); v0f = np.floor(iv)
            u0 = u0f.astype(np.int64); v0 = v0f.astype(np.int64)
            vx = ((u0 >= 0) & (u0 < WUP)) | ((u0 + 1 >= 0) & (u0 + 1 < WUP))
            vy = ((v0 >= 0) & (v0 < HUP)) | ((v0 + 1 >= 0) & (v0 + 1 < HUP))
            valid = vx & vy
            umn, umx, anyv = _quad_spans(u0, valid)
            vmn, vmx, _ = _quad_spans(v0, valid)
            du = np.where(anyv, umx - umn, 0).max()
            dv = np.where(anyv, vmx - vmn, 0).max()
            cand = dict(otr=otr, st=st, iu=iu, iv=iv, u0=u0, v0=v0,
                        valid=valid, du=int(du), dv=int(dv))
            if du <= NMAJ - 2 and dv <= NCELL - 2:
                score = (du, dv)
                if best is None or score < best[0]:
                    best = (score, cand)
    assert best is not None, "no feasible D4 transform for quad gather"
    return best[1]


def host_prep(images, u_flip, u_rot90, u_int, z_scale, u_rot1, z_aniso, u_rot2,
              z_frac, hz_geom):
    imgs = np.asarray(images, F32)
    f = np.asarray(hz_geom, F32)
    n = imgs.shape[0]
    G = _compose_G(np.asarray(u_flip, F32), np.asarray(u_rot90, F32),
                   np.asarray(u_int, F32), np.asarray(z_scale, F32),
                   np.asarray(u_rot1, F32), np.asarray(z_aniso, F32),
                   np.asarray(u_rot2, F32), np.asarray(z_frac, F32))

    plans = [_plan_sample(G[i, :2, :]) for i in range(n)]

    # bbox per sample (valid pixels, +1 neighbor, +1 exclusive)
    for p in plans:
        uu = p["u0"][p["valid"]]; vv = p["v0"][p["valid"]]
        if uu.size == 0:
            p["bbox"] = (0, 8, 0, 8)
        else:
            p["bbox"] = (int(uu.min()), int(uu.max()) + 2,
                         int(vv.min()), int(vv.max()) + 2)
    UB = max(p["bbox"][1] - p["bbox"][0] for p in plans)
    VB = max(p["bbox"][3] - p["bbox"][2] for p in plans)
    UB = max(UB, NMAJ + 1)
    VB = max(VB, NCELL + 1)

    # upsample bands (per sample; union shapes)
    bands = []
    for p in plans:
        u_org, _, v_org, _ = p["bbox"]
        Wv, jv0, jv1 = _upsample_band(v_org, v_org + VB, HUP, f)
        Wh, jh0, jh1 = _upsample_band(u_org, u_org + UB + NMAJ - 1, WUP, f)
        bands.append((Wv, jv0, jv1, Wh, jh0, jh1))
    HJ = max(b[2] - b[1] for b in bands)
    WN = max(b[5] - b[4] for b in bands)
    KTJ = (HJ + 127) // 128
    KTN = (WN + 127) // 128
    HJp = KTJ * 128

    VT = (VB + 127) // 128
    UT = (UB + 127) // 128
    vt_span = []
    for vt in range(VT):
        klo, khi = KTJ, -1
        for i in range(n):
            Wv = bands[i][0]
            sl = Wv[:, vt * 128: min((vt + 1) * 128, VB)]
            nz = np.nonzero(np.any(sl != 0, axis=1))[0]
            if nz.size:
                klo = min(klo, int(nz.min()) // 128)
                khi = max(khi, int(nz.max()) // 128)
        if khi < 0:
            klo, khi = 0, 0
        vt_span.append((klo, khi))
    ut_span = []
    for ut in range(UT):
        klo, khi = KTN, -1
        for i in range(n):
            Wh = bands[i][3]
            sl = Wh[:, ut * 128: min(ut * 128 + 128 + NMAJ - 1, UB + NMAJ - 1)]
            nz = np.nonzero(np.any(sl != 0, axis=1))[0]
            if nz.size:
                klo = min(klo, int(nz.min()) // 128)
                khi = max(khi, int(nz.max()) // 128)
        if khi < 0:
            klo, khi = 0, 0
        ut_span.append((klo, khi))

    Dv = _downsample_band(H, HO, f)
    Dh = _downsample_band(W, WO, f)
    # s_sb stores x in j-major order (slot = j*QX + q for x = 4q+j); permute
    # Dh rows to match so the downsample contracts the permuted axis directly.
    perm = (np.arange(WO).reshape(QX, 4).T).reshape(-1)
    Dh = np.ascontiguousarray(Dh[perm])

    in_maps = []
    my = mx = W - 1
    for i in range(n):
        p = plans[i]
        u_org, _, v_org, _ = p["bbox"]
        Wv, jv0, jv1, Wh, jh0, jh1 = bands[i]

        wv = np.zeros((HJp, VB), F32)
        wv[: jv1 - jv0, : Wv.shape[1]] = Wv * F32(4.0)
        wh = np.zeros((KTN * 128, UB + NMAJ - 1), F32)
        wh[: jh1 - jh0, : Wh.shape[1]] = Wh

        img = imgs[i] if p["st"] == 0 else np.ascontiguousarray(
            imgs[i].transpose(0, 2, 1))
        P = np.pad(img, ((0, 0), (my, my), (mx, mx)), mode="reflect")
        pc = np.zeros((3, HJp, WN), F32)
        pc[:, : jv1 - jv0, : jh1 - jh0] = P[:, jv0:jv1, jh0:jh1]

        # quad planning
        u0 = p["u0"] - u_org
        v0 = p["v0"] - v_org
        valid = p["valid"]
        wx = (p["iu"] - np.floor(p["iu"])).astype(F32)
        wy = (p["iv"] - np.floor(p["iv"])).astype(F32)
        w4 = np.empty((HO, WO, 2, 2), F32)   # [dv, du]
        w4[..., 0, 0] = (1 - wx) * (1 - wy)
        w4[..., 0, 1] = wx * (1 - wy)
        w4[..., 1, 0] = (1 - wx) * wy
        w4[..., 1, 1] = wx * wy
        w4[~valid] = 0.0

        umn, umx, anyv = _quad_spans(u0, valid)
        vmn, vmx, _ = _quad_spans(v0, valid)
        mb = np.where(anyv, np.minimum(umn, UB - NMAJ), 0).astype(np.int64)
        tb = np.where(anyv, np.minimum(vmn, VB - NCELL), 0).astype(np.int64)
        assert (np.where(anyv, umx, 0) <= mb + NMAJ - 2).all()
        assert (np.where(anyv, vmx, 0) <= tb + NCELL - 2).all()
        idxq = (mb * VB + tb).astype(np.int32)            # [HO, QX]

        # weights [HO, QX, 4, NCELL, NMAJ]
        wq = np.zeros((HO, QX, 4, NCELL, NMAJ), F32)
        u0q = u0.reshape(HO, QX, 4); v0q = v0.reshape(HO, QX, 4)
        vq = valid.reshape(HO, QX, 4)
        w4q = w4.reshape(HO, QX, 4, 2, 2)
        for dv in range(2):
            for du in range(2):
                cell = v0q + dv - tb[:, :, None]
                maj = u0q + du - mb[:, :, None]
                cell = np.where(vq, cell, 0)
                maj = np.where(vq, maj, 0)
                assert (cell >= 0).all() and (cell < NCELL).all()
                assert (maj >= 0).all() and (maj < NMAJ).all()
                np.put_along_axis(
                    wq.reshape(HO, QX, 4, NSLOT),
                    (cell * NMAJ + maj)[..., None],
                    w4q[:, :, :, dv, du][..., None], axis=3)

        idx_m = idxq[: YTM * 128].reshape(YTM, 128, QX)
        wq_m = wq[: YTM * 128].reshape(YTM, 128, QX, 4, NCELL, NMAJ).astype(F16)

        # tail rows 512..523 packed [TP=96, TK]
        idx_t = np.zeros((TP, TK), np.int32)
        wq_t = np.zeros((TP, TK, 4, NCELL, NMAJ), F16)
        for g in range(TGRP):
            c0 = g * TK
            nc_ = min(TK, QX - c0)
            if nc_ <= 0:
                continue
            idx_t[g * 32: g * 32 + TY, :nc_] = idxq[YTM * 128:, c0:c0 + nc_]
            wq_t[g * 32: g * 32 + TY, :nc_] = wq[YTM * 128:, c0:c0 + nc_]

        in_maps.append({
            "pc": pc,
            "wv": wv,
            "wh": wh,
            "dv": Dv,
            "dh": Dh,
            "idxq": idx_m,
            "wq": wq_m,
            "idxt": idx_t,
            "wqt": wq_t,
        })

    dims = dict(UB=UB, VB=VB, HJ=HJ, WN=WN, KTJ=KTJ, KTN=KTN, VT=VT, UT=UT,
                vt_span=vt_span, ut_span=ut_span)
    return dims, in_maps, [(p["otr"], p["st"]) for p in plans]


# ----------------------------------------------------------------------------
# device program
# ----------------------------------------------------------------------------

def build_program(dims):
    UB, VB = dims["UB"], dims["VB"]
    WN = dims["WN"]
    KTJ, KTN = dims["KTJ"], dims["KTN"]
    VT, UT = dims["VT"], dims["UT"]
    vt_span, ut_span = dims["vt_span"], dims["ut_span"]

    cdt = mybir.dt.float16
    NCHUNK = 512

    nc = bacc.Bacc(trn_type="TRN2")

    pc_in = nc.dram_tensor("pc", [3, KTJ * 128, WN], mybir.dt.float32, kind="ExternalInput")
    wv_in = nc.dram_tensor("wv", [KTJ * 128, VB], mybir.dt.float32, kind="ExternalInput")
    wh_in = nc.dram_tensor("wh", [KTN * 128, UB + NMAJ - 1], mybir.dt.float32, kind="ExternalInput")
    dv_in = nc.dram_tensor("dv", [HO, 256], mybir.dt.float32, kind="ExternalInput")
    dh_in = nc.dram_tensor("dh", [WO, 256], mybir.dt.float32, kind="ExternalInput")
    idxq_in = nc.dram_tensor("idxq", [YTM, 128, QX], mybir.dt.int32, kind="ExternalInput")
    wq_in = nc.dram_tensor("wq", [YTM, 128, QX, 4, NCELL, NMAJ], mybir.dt.float16, kind="ExternalInput")
    idxt_in = nc.dram_tensor("idxt", [TP, TK], mybir.dt.int32, kind="ExternalInput")
    wqt_in = nc.dram_tensor("wqt", [TP, TK, 4, NCELL, NMAJ], mybir.dt.float16, kind="ExternalInput")
    out = nc.dram_tensor("out", [3, H, W], mybir.dt.float32, kind="ExternalOutput")

    tbl = nc.dram_tensor("tbl", [UB * VB, 12], mybir.dt.float16,
                         kind="ExternalOutput" if os.environ.get("AUG_DBG") else "Internal")
    if os.environ.get("AUG_DBG"):
        dbg_g = nc.dram_tensor("dbg_g", [128, QX, NCELL, 12], mybir.dt.float16, kind="ExternalOutput")
        dbg_s = nc.dram_tensor("dbg_s", [128, WO], mybir.dt.float16, kind="ExternalOutput")

    def chunks(total, step=NCHUNK):
        res = []
        o = 0
        while o < total:
            res.append((o, min(step, total - o)))
            o += step
        return res

    with TileContext(nc) as tc:
        with (
            tc.tile_pool(name="const", bufs=1) as constp,
            tc.tile_pool(name="wband", bufs=1) as wbandp,
            tc.tile_pool(name="pc", bufs=1) as pcp,
            tc.tile_pool(name="b", bufs=2) as bp,
            tc.tile_pool(name="bt", bufs=1) as btp,
            tc.tile_pool(name="big", bufs=2) as bigp,
            tc.tile_pool(name="idxw", bufs=1) as idxwp,
            tc.tile_pool(name="s", bufs=1) as sp,
            tc.tile_pool(name="ds", bufs=1) as dsp,
            tc.tile_pool(name="psmm", bufs=2, space="PSUM") as psmm,
            tc.tile_pool(name="pstr", bufs=2, space="PSUM") as pstr,
        ):
            ident = constp.tile([128, 128], cdt, tag="ident", name="ident")
            make_identity(nc, ident[:])

            # --- bands (f32 -> f16 via gpsimd cast DMA) ---
            wv_t = []
            for kt in range(KTJ):
                t = wbandp.tile([128, VB], cdt, tag=f"wvh{kt}", name=f"wv{kt}")
                nc.gpsimd.dma_start(out=t[:], in_=wv_in[kt * 128:(kt + 1) * 128, :])
                wv_t.append(t)
            wh_t = []
            for kt in range(KTN):
                t = wbandp.tile([128, UB + NMAJ - 1], cdt, tag=f"wvh{kt}", name=f"wh{kt}")
                nc.gpsimd.dma_start(out=t[:], in_=wh_in[kt * 128:(kt + 1) * 128, :])
                wh_t.append(t)

            # downsample band loads early: Pool engine is idle pre-gather
            YT0 = YTM + 1
            dv_t = []
            for kt in range(YT0):
                py = min(128, HO - kt * 128)
                t = dsp.tile([128, 256], cdt, tag=f"dv{kt}", name=f"dvt{kt}")
                nc.gpsimd.dma_start(out=t[:py, :], in_=dv_in[kt * 128: kt * 128 + py, :])
                dv_t.append(t)
            dh_t = []
            for kt in range(YT0):
                px = min(128, WO - kt * 128)
                t = dsp.tile([128, 256], cdt, tag=f"dh{kt}", name=f"dht{kt}")
                nc.gpsimd.dma_start(out=t[:px, :], in_=dh_in[kt * 128: kt * 128 + px, :])
                dh_t.append(t)

            # --- phase 1: vertical upsample + transpose ---
            bt_sb = [[btp.tile([128, VB], cdt, tag=f"bt{c}_{nt}", name=f"bt{c}_{nt}")
                      for nt in range(KTN)] for c in range(3)]
            pn_last = WN - (KTN - 1) * 128
            if pn_last < 128:
                for c in range(3):
                    nc.vector.memset(bt_sb[c][KTN - 1][:], 0.0)
            for c in range(3):
                pc_t = []
                for kt in range(KTJ):
                    t = pcp.tile([128, WN], cdt, tag=f"pc{kt}_{c%2}", name=f"pct{kt}")
                    nc.gpsimd.dma_start(out=t[:], in_=pc_in[c, kt * 128:(kt + 1) * 128, :])
                    pc_t.append(t)
                for vt in range(VT):
                    pv = min(128, VB - vt * 128)
                    klo, khi = vt_span[vt]
                    b_t = bp.tile([128, WN], cdt, tag="b", name="b_t")
                    for (wo, wn) in chunks(WN):
                        ps = psmm.tile([128, NCHUNK], mybir.dt.float32,
                                       space="PSUM", tag="mm", name="ps")
                        for ki, kt in enumerate(range(klo, khi + 1)):
                            nc.tensor.matmul(
                                out=ps[:pv, :wn],
                                lhsT=wv_t[kt][:, vt * 128: vt * 128 + pv],
                                rhs=pc_t[kt][:, wo: wo + wn],
                                start=(ki == 0), stop=(kt == khi),
                            )
                        nc.scalar.copy(out=b_t[:pv, wo: wo + wn], in_=ps[:pv, :wn])
                    for nt in range(KTN):
                        pn = min(128, WN - nt * 128)
                        pst = pstr.tile([128, 128], cdt, space="PSUM", tag="tr", name="pst")
                        nc.tensor.transpose(
                            out=pst[:pn, :pv],
                            in_=b_t[:pv, nt * 128: nt * 128 + pn],
                            identity=ident[:pv, :pv],
                        )
                        if nt % 2 == 0:
                            nc.scalar.copy(
                                out=bt_sb[c][nt][:pn, vt * 128: vt * 128 + pv],
                                in_=pst[:pn, :pv],
                            )
                        else:
                            nc.vector.tensor_copy(
                                out=bt_sb[c][nt][:pn, vt * 128: vt * 128 + pv],
                                in_=pst[:pn, :pv],
                            )

            # --- phase 2: horizontal upsample x4 majors + table assembly ---
            tbl3 = tbl.rearrange("(u v) x -> u v x", v=VB)
            for ut in range(UT):
                pu = min(128, UB - ut * 128)
                klo, khi = ut_span[ut]
                tasm = bigp.tile([128, VB, 12], cdt, tag="big", name="tasm")
                for c in range(3):
                    for du in range(NMAJ):
                        for (vo, vn) in chunks(VB):
                            ps = psmm.tile([128, NCHUNK], mybir.dt.float32,
                                           space="PSUM", tag="mm", name="ps")
                            for ki, kt in enumerate(range(klo, khi + 1)):
                                nc.tensor.matmul(
                                    out=ps[:pu, :vn],
                                    lhsT=wh_t[kt][:, ut * 128 + du: ut * 128 + du + pu],
                                    rhs=bt_sb[c][kt][:, vo: vo + vn],
                                    start=(ki == 0), stop=(kt == khi),
                                )
                            if du % 2 == 0:
                                nc.scalar.copy(
                                    out=tasm[:pu, vo: vo + vn, NMAJ * c + du],
                                    in_=ps[:pu, :vn],
                                )
                            else:
                                nc.vector.tensor_copy(
                                    out=tasm[:pu, vo: vo + vn, NMAJ * c + du],
                                    in_=ps[:pu, :vn],
                                )
                nc.sync.dma_start(out=tbl3[ut * 128: ut * 128 + pu], in_=tasm[:pu])

            # --- phase 3: quad gathers + combine ---
            s_sb = [[sp.tile([128, WO], cdt, tag=f"s{c}_{yt}", name=f"s{c}_{yt}")
                     for yt in range(YTM + 1)] for c in range(3)]
            # --- tail rows ---
            idx4 = idxwp.tile([TP, TK], mybir.dt.int32, tag="idx4", name="idx4")
            nc.sync.dma_start(out=idx4[:], in_=idxt_in[:, :])
            w4_t = idxwp.tile([TP, TK, 4, NCELL, NMAJ], mybir.dt.float16,
                              tag="wgt4", name="w4_t")
            nc.sync.dma_start(out=w4_t[:], in_=wqt_in[:, :])
            g4_t = idxwp.tile([TP, TK, NCELL, 12], mybir.dt.float16, tag="g4", name="g4_t")
            for k in range(TK):
                nc.gpsimd.indirect_dma_start(
                    out=g4_t[:, k].rearrange("p t x -> p (t x)"), out_offset=None,
                    in_=tbl[:],
                    in_offset=bass.IndirectOffsetOnAxis(ap=idx4[:, k:k + 1], axis=0),
                )
            tmp4 = idxwp.tile([TP, TK, NSLOT], mybir.dt.float16,
                              tag="ctmp4", name="tmp4")
            gc4 = idxwp.tile([TP, TK, NSLOT], mybir.dt.float16,
                             tag="gc4", name="gc4")
            s4 = idxwp.tile([TP, TK * 4], cdt, tag="s4", name="s4")
            for c in range(3):
                nc.scalar.copy(
                    out=gc4[:].rearrange("p k (t m) -> p k t m", m=NMAJ),
                    in_=g4_t[:, :, :, NMAJ * c: NMAJ * c + NMAJ])
                s4view = s4[:].rearrange("p (j k) -> p j k", k=TK)
                for j in range(4):
                    nc.vector.tensor_tensor(
                        out=tmp4[:],
                        in0=w4_t[:, :, j].rearrange("p k t m -> p k (t m)"),
                        in1=gc4[:],
                        op=mybir.AluOpType.mult)
                    with nc.allow_low_precision(reason="24-tap sum, tol 2e-2"):
                        nc.vector.tensor_reduce(
                            out=s4view[:, j, :],
                            in_=tmp4[:],
                            op=mybir.AluOpType.add,
                            axis=mybir.AxisListType.X)
                s4v = s4[:].rearrange("p (j k) -> p j k", k=TK)
                for j in range(4):
                    for g in range(TGRP):
                        nk = min(TK, QX - g * TK)
                        if nk <= 0:
                            continue
                        nc.scalar.copy(
                            out=s_sb[c][YTM][:TY, j * QX + g * TK: j * QX + g * TK + nk],
                            in_=s4v[g * 32: g * 32 + TY, j, :nk],
                        )

            for yt in range(YTM):
                idx_t = idxwp.tile([128, QX], mybir.dt.int32, tag=f"idx{yt%2}", name="idx_t")
                nc.sync.dma_start(out=idx_t[:], in_=idxq_in[yt])
                w_t = idxwp.tile([128, QX, 4, NCELL, NMAJ], mybir.dt.float16,
                                 tag="wgt", name="w_t")
                nc.sync.dma_start(out=w_t[:], in_=wq_in[yt])
                g_t = idxwp.tile([128, QX, NCELL, 12], mybir.dt.float16,
                                 tag=f"g{yt%2}", name="g_t")
                for q in range(QX):
                    nc.gpsimd.indirect_dma_start(
                        out=g_t[:, q].rearrange("p t x -> p (t x)"), out_offset=None,
                        in_=tbl[:],
                        in_offset=bass.IndirectOffsetOnAxis(ap=idx_t[:, q:q + 1], axis=0),
                    )
                if os.environ.get("AUG_DBG") and yt == 0:
                    nc.sync.dma_start(out=dbg_g[:], in_=g_t[:])
                tmp = idxwp.tile([128, QX, NSLOT], mybir.dt.float16,
                                 tag="ctmp", name="ctmp")
                gc = idxwp.tile([128, QX, NSLOT], mybir.dt.float16,
                                tag="gc", name="gc")
                for c in range(3):
                    # expand channel slice into contiguous [128, QX, NSLOT]
                    nc.scalar.copy(
                        out=gc[:].rearrange("p q (t m) -> p q t m", m=NMAJ),
                        in_=g_t[:, :, :, NMAJ * c: NMAJ * c + NMAJ])
                    sview = s_sb[c][yt][:, :].rearrange("p (j q) -> p j q", q=QX)
                    for j in range(4):
                        nc.vector.tensor_tensor(
                            out=tmp[:],
                            in0=w_t[:, :, j].rearrange("p q t m -> p q (t m)"),
                            in1=gc[:],
                            op=mybir.AluOpType.mult)
                        with nc.allow_low_precision(reason="24-tap sum, tol 2e-2"):
                            nc.vector.tensor_reduce(
                                out=sview[:, j, :],
                                in_=tmp[:],
                                op=mybir.AluOpType.add,
                                axis=mybir.AxisListType.X)

            if os.environ.get("AUG_DBG"):
                nc.sync.dma_start(out=dbg_s[:], in_=s_sb[0][0][:])
            # --- phase 4: downsample (dv/dh loads hoisted to top) ---
            YT = YTM + 1

            for c in range(3):
                v_sb = [dsp.tile([128, WO], cdt, tag=f"v{t_}", name=f"v_sb{t_}") for t_ in range(2)]
                for Yt in range(2):
                    ylo = 2 * (Yt * 128) + 1
                    yhi = 2 * (Yt * 128 + 127) + 1 + TAPS - 1
                    klo, khi = ylo // 128, min(yhi, HO - 1) // 128
                    for (xo, xn) in ((0, 262), (262, 262)):
                        ps = psmm.tile([128, NCHUNK], mybir.dt.float32,
                                       space="PSUM", tag="mm", name="ps")
                        for ki, kt in enumerate(range(klo, khi + 1)):
                            py = min(128, HO - kt * 128)
                            nc.tensor.matmul(
                                out=ps[:, :xn],
                                lhsT=dv_t[kt][:py, Yt * 128: Yt * 128 + 128],
                                rhs=s_sb[c][kt][:py, xo: xo + xn],
                                start=(ki == 0), stop=(kt == khi),
                            )
                        nc.scalar.copy(out=v_sb[Yt][:, xo: xo + xn], in_=ps[:, :xn])
                vt_sb = [dsp.tile([128, 256], cdt, tag=f"vt{t_}", name=f"vt_sb{t_}") for t_ in range(YT)]
                for xt in range(YT):
                    px = min(128, WO - xt * 128)
                    for Yt in range(2):
                        pst = pstr.tile([128, 128], cdt, space="PSUM", tag="tr", name="pst")
                        nc.tensor.transpose(
                            out=pst[:px, :],
                            in_=v_sb[Yt][:, xt * 128: xt * 128 + px],
                            identity=ident[:],
                        )
                        nc.scalar.copy(
                            out=vt_sb[xt][:px, Yt * 128: Yt * 128 + 128],
                            in_=pst[:px, :],
                        )
                ot_sb = [dsp.tile([128, 256], cdt, tag=f"ot{t_}", name=f"ot_sb{t_}") for t_ in range(2)]
                for Xt in range(2):
                    # x axis is j-major permuted: contributions span all tiles
                    klo, khi = 0, YT - 1
                    ps = psmm.tile([128, NCHUNK], mybir.dt.float32,
                                   space="PSUM", tag="mm", name="ps")
                    for ki, kt in enumerate(range(klo, khi + 1)):
                        px = min(128, WO - kt * 128)
                        nc.tensor.matmul(
                            out=ps[:, :256],
                            lhsT=dh_t[kt][:px, Xt * 128: Xt * 128 + 128],
                            rhs=vt_sb[kt][:px, :],
                            start=(ki == 0), stop=(kt == khi),
                        )
                    nc.scalar.copy(out=ot_sb[Xt][:, :], in_=ps[:, :256])
                for Yt in range(2):
                    o_sb = dsp.tile([128, 256], mybir.dt.float32, tag="o", name="o_sb", bufs=2)
                    for Xt in range(2):
                        pst = pstr.tile([128, 128], cdt, space="PSUM", tag="tr", name="pst")
                        nc.tensor.transpose(
                            out=pst[:, :],
                            in_=ot_sb[Xt][:, Yt * 128: Yt * 128 + 128],
                            identity=ident[:],
                        )
                        nc.scalar.copy(
                            out=o_sb[:, Xt * 128: Xt * 128 + 128],
                            in_=pst[:, :],
                        )
                    nc.sync.dma_start(
                        out=out[c, Yt * 128: Yt * 128 + 128, :], in_=o_sb[:])

    nc.compile()
    return nc


# ----------------------------------------------------------------------------
# entry point
# ----------------------------------------------------------------------------

def run(inputs, trace=False, **spmd_kwargs):
    dims, in_maps, xforms = host_prep(**inputs)
    nc = build_program(dims)

    from concourse.bass_utils import run_bass_kernel_spmd
    res = run_bass_kernel_spmd(nc, in_maps, core_ids=list(range(N_CORES)),
                               trace=trace, **spmd_kwargs)
    outs = []
    for i in range(N_CORES):
        o = np.asarray(res.results[i]["out"], np.float32)
        if xforms[i][0]:   # output was gathered transposed
            o = np.ascontiguousarray(o.transpose(0, 2, 1))
        outs.append(o)
    return np.stack(outs), res


def kernel(**inputs):
    out, _ = run(inputs)
    return out
